# revision 27
# baseline (speedup 1.0000x reference)
"""Dense dot-product attention with key-length masking on 8 Trainium2 cores.

Problem: q,k,v [16, 2048, 128] fp32, valid_lens [16,1] int32.
  out = softmax(mask(q@k.T/sqrt(d))) @ v   (masked keys -> -1e6 before softmax)

The kernel is PSUM-drain bound: every score element must leave PSUM through
ScalarE (ACT) or VectorE (DVE) - the only engines that read PSUM.  The design
splits that drain and everything downstream across all five engines:

- S^T tiles (keys on partitions) from PE; fp16 operands, host pre-transposed.
- ~55% of key tiles drain via ScalarE exp(); the rest via DVE Schraudolph
  exp: one tensor_scalar computes round(S*sc_k + bi_k) into int16 (RNE,
  hardware-verified) whose bitcast IS fp16 exp(S/sqrt(d)) to ~3%.  Both
  paths mask through per-partition scale/bias vectors (masked keys land on
  exp(-30) resp. +0.0 exactly), so the ACT/DVE split is a pure
  load-balance knob and the tiles interleave so both engines drain
  concurrently.  Softmax renormalization cancels the shared Schraudolph
  bias (C chosen to zero its mean vs true exp).
- O^T accumulates over key tiles with V stationary, E moving (fp16).
- Softmax denominators: E tiles live in one contiguous [128, t, QCH] tile
  per slot; one level of in-place pair-sums (GpSimd for the largest slot -
  it is otherwise idle but its SBUF port contends with DVE - DVE for the
  rest) then ONE batched DMA per slot ships the partials; the host
  finishes the reduction and the divide + transpose.  oT is copied
  PSUM->SBUF as fp16 by ScalarE.  This keeps total DMA under the per-core
  bandwidth while freeing the drain engines from a full reduction tree.
- Inputs are packed per slot into one contiguous [128, X] fp16 DRAM buffer
  (qT | kT | v-permuted), all input DMAs pre-issued up front on the sync
  queue only (outputs go to other queues), so no data-dependent DMA ever
  head-of-line blocks an input transfer; slot 0 is split into pieces on
  distinct queues so the pipeline ignites early.
- HAM warm-up: dummy bf16 matmuls run while the input DMAs stream.

Work distribution (valid_lens-aware, single SPMD program): since the host
sums partial numerators/denominators anyway, a (batch, query-half) unit can
split at any key-tile boundary across cores.  A small solver picks per-slot
sizes (shared across cores, SPMD) and packs segments to reach the ideal
ceil(sum(ceil(L/128))/8) key tiles per core (32 here vs 37 for unsplit
grouping).  Slots run largest-first (serial GpSimd pairing starts early),
smallest last (its drain + esum DMA is the tail).  L==0 batches are pure
uniform softmax = mean(v), computed on the host.
"""

import math
import sys
import types

import numpy as np

import concourse.bass as bass
import concourse.mybir as mybir
import concourse.tile as tile
from concourse import bacc
from concourse.bass_utils import run_bass_kernel_spmd

B, Q, K, D = 16, 2048, 2048, 128
NCORES = 8
QCH = 1024         # queries per work unit
UNITS = B * (Q // QCH)
NSLOT = UNITS // NCORES
MM_N = 512         # moving-operand free dim per matmul
KT = K // 128      # max key tiles
SCALE = 1.0 / math.sqrt(D)
LOG2E = 1.4426950408889634
SCHC = 0.0574      # Schraudolph shift: zero-mean vs true exp under softmax
WARMUP_MMS = 4     # dummy matmuls to lift the PE HAM clock-gate
GP_FRAC = 0.42     # fraction of the denominator tree on GpSimd

F32 = mybir.dt.float32
F16 = mybir.dt.float16
I16 = mybir.dt.int16
BF16 = mybir.dt.bfloat16


def _install_hook_stub():
    """bass_utils' axon trace path imports antenv.axon_hooks, which is not
    shipped in this container.  Provide a no-op stub so an ambient
    BASS_TRACE=1 doesn't crash; test harnesses may overwrite the hook."""
    if "antenv.axon_hooks" in sys.modules:
        return
    mod = types.ModuleType("antenv.axon_hooks")
    _hook = [None]
    mod.set_axon_ntff_profile_hook = lambda h: _hook.__setitem__(0, h)
    mod.get_axon_ntff_profile_hook = lambda: _hook[0]
    sys.modules["antenv.axon_hooks"] = mod


_install_hook_stub()

_build_cache = {}
last_result = None  # BassKernelResults of the most recent run (for harnesses)


def _tree_split(t, n_act):
    """Assign each of the t E tiles of a slot to the GpSimd or DVE partial
    accumulator, spreading GpSimd's (slow, serial) share across the slot."""
    gp_cnt = min(t - 0, max(0, round(GP_FRAC * t)))
    gp = set()
    acc = 0.0
    for i in range(t):
        acc += gp_cnt / t
        if acc >= 1.0 and len(gp) < gp_cnt:
            acc -= 1.0
            gp.add(i)
    return gp


def _plan_slots(needs):
    """Partition the work (per-unit key-tile counts, units splittable at any
    key-tile boundary because the host sums partial numerators/denominators)
    into NCORES x m segments with per-slot sizes shared across cores (SPMD).
    Returns (sizes, assign) where assign[core][slot] = (b, h, off, len) or
    None; minimizes sum(sizes) (everything scales with it)."""
    total = sum(n for n, _, _ in needs)
    lo = -(-total // NCORES)
    nmax = max(n for n, _, _ in needs)

    def try_pack(sizes):
        bins = []  # (cap, slot_idx, core) - core assignment is arbitrary
        used = [[0] * NCORES for _ in sizes]
        rem = sorted([[n, b, h, 0] for n, b, h in needs], reverse=True)
        segs = [[None] * len(sizes) for _ in range(NCORES)]
        free = []
        for j, t in enumerate(sizes):
            for c in range(NCORES):
                free.append([t, j, c])
        for _ in range(10000):
            rem = [r for r in rem if r[0] > 0]
            if not rem:
                return segs
            rem.sort(key=lambda r: -r[0])
            u = rem[0]
            n = u[0]
            if not free:
                return None
            exact = [f for f in free if f[0] <= n]
            if exact:
                f = max(exact, key=lambda f: f[0])
            else:
                f = min(free, key=lambda f: f[0])
            free.remove(f)
            ln = min(n, f[0])
            segs[f[2]][f[1]] = (u[1], u[2], u[3], ln)
            u[3] += ln
            u[0] -= ln
        return None

    best = None
    sizes_list = []

    def enum(parts, remaining, maxp):
        if len(parts) > 6:
            return
        if lo <= sum(parts) <= lo + 4 and parts:
            sizes_list.append(tuple(parts))
        if sum(parts) >= lo + 4:
            return
        for p in range(min(maxp, lo + 4 - sum(parts)), 0, -1):
            parts.append(p)
            enum(parts, remaining, p)
            parts.pop()

    enum([], total, min(16, nmax + 2))
    sizes_list.sort(key=lambda s: (sum(s), len(s)))
    for sizes in sizes_list:
        if sizes[0] < 2:
            continue
        segs = try_pack(list(sizes))
        if segs is not None:
            return list(sizes), segs
    # fallback: one slot per unit chunk of the old grouping shape
    raise RuntimeError("no feasible slot plan")


def _build(trips, act_sets, pair_eng):
    """One SPMD program: slot j processes trips[j] key tiles of one unit.
    Tiles in act_sets[j] drain via ScalarE exp(), the rest via DVE
    Schraudolph; both paths mask via per-partition scale/bias vectors, so
    the split is a pure load-balance knob and the two drain engines run
    concurrently.  E tiles live in one contiguous [128, t, QCH] tile per
    slot; the denominator is shipped to the host as one level of pair-sums
    (engine per slot in pair_eng: 'v'=DVE, 'g'=GpSimd, ''=ship raw)."""
    nc = bacc.Bacc(num_devices=NCORES)

    nslot = len(trips)
    t_all = list(trips)
    xlens = [QCH + 2 * 128 * t for t in t_all]
    nship = [t if e == "" else (t + 1) // 2 for t, e in zip(t_all, pair_eng)]
    inbs = [
        nc.declare_dram_parameter(f"inb{s}", [128, xlens[s]], F16, isOutput=False)
        for s in range(nslot)
    ]
    scbi = nc.declare_dram_parameter("scbi", [128, nslot * 4 * KT], F32, isOutput=False)
    oT = nc.declare_dram_parameter("oT", [nslot, 128, QCH], F16, isOutput=True)
    esums = [
        nc.declare_dram_parameter(f"esum{s}", [128, nship[s], QCH], F16, isOutput=True)
        for s in range(nslot)
    ]

    with tile.TileContext(nc) as tc:
        with (
            tc.tile_pool(name="small", bufs=2) as small,
            tc.tile_pool(name="inputs", bufs=1) as inpool,
            tc.tile_pool(name="epool", bufs=1) as epool,
            tc.tile_pool(name="sps", bufs=3, space="PSUM") as pspool,
            tc.tile_pool(name="oacc", bufs=1, space="PSUM") as psacc,
        ):
            # --- HAM warm-up: dummy bf16 matmuls while input DMAs stream ---
            wsrc = small.tile([128, MM_N], BF16)
            nc.gpsimd.memset(wsrc[:], 1.0)

            # pre-issue every input DMA up front: the sync queue carries only
            # inputs, so nothing data-dependent ever blocks an input transfer
            inb_tiles = []
            for s in range(nslot):
                inb = inpool.tile([128, xlens[s]], F16, tag=f"inb{s}")
                inb_tiles.append(inb)
                kbase, vbase = QCH, QCH + 128 * t_all[s]
                if s == 0:
                    # ignition pieces on distinct queues for parallel issue
                    nc.sync.dma_start(out=inb[:, 0:MM_N], in_=inbs[s][:, 0:MM_N])
                    nc.scalar.dma_start(out=inb[:, MM_N:QCH], in_=inbs[s][:, MM_N:QCH])
                    nc.gpsimd.dma_start(
                        out=inb[:, kbase : kbase + 128],
                        in_=inbs[s][:, kbase : kbase + 128],
                    )
                    nc.gpsimd.dma_start(
                        out=inb[:, vbase : vbase + 128],
                        in_=inbs[s][:, vbase : vbase + 128],
                    )
                    # early tiles first: kT/v tiles 1-4, then the rest
                    kcut = min(kbase + 5 * 128, vbase)
                    vcut = min(vbase + 5 * 128, xlens[s])
                    nc.sync.dma_start(
                        out=inb[:, kbase + 128 : kcut],
                        in_=inbs[s][:, kbase + 128 : kcut],
                    )
                    nc.sync.dma_start(
                        out=inb[:, vbase + 128 : vcut],
                        in_=inbs[s][:, vbase + 128 : vcut],
                    )
                    if kcut < vbase:
                        nc.sync.dma_start(
                            out=inb[:, kcut:vbase], in_=inbs[s][:, kcut:vbase]
                        )
                    if vcut < xlens[s]:
                        nc.sync.dma_start(
                            out=inb[:, vcut:], in_=inbs[s][:, vcut:]
                        )
                else:
                    nc.sync.dma_start(out=inb[:], in_=inbs[s][:])
            sc_sb = small.tile([128, nslot * 4 * KT], F32)
            nc.scalar.dma_start(out=sc_sb[:], in_=scbi[:])

            for w in range(WARMUP_MMS):
                if w % 2 == 0:
                    wps = pspool.tile([128, QCH], F32, tag="s")
                nc.tensor.matmul(
                    wps[:, (w % 2) * MM_N : (w % 2) * MM_N + MM_N],
                    wsrc[:, :128],
                    wsrc[:],
                    start=True,
                    stop=True,
                    skip_group_check=True,
                )

            # per-slot contexts (E tiles and O accumulators allocated up
            # front; deps attach at instruction emission, not allocation)
            ets = []
            o_pss = []
            for s in range(nslot):
                et_s = epool.tile([128, t_all[s], QCH], F16, tag=f"e{s}")
                ets.append(et_s)
                o_ps = psacc.tile([128, QCH], F32, tag="o")
                o_pss.append(o_ps)

            # software-pipeline the PE queue globally: the O matmul of tile
            # (s, i) waits on E_(s,i) (a drain-engine product), so issue the
            # S matmuls LOOK tiles ahead of it - across slot boundaries too,
            # so the PE keeps streaming while a slot's tail drains and its
            # oT copy runs.
            LOOK = 3  # = sps pool depth
            sps_ref = {}

            def emit_s(s, i):
                s_ps = pspool.tile([128, QCH], F32, tag="s")
                sps_ref[(s, i)] = s_ps
                inb = inb_tiles[s]
                for h in range(QCH // MM_N):
                    nc.tensor.matmul(
                        s_ps[:, bass.ts(h, MM_N)],
                        inb[:, QCH + i * 128 : QCH + (i + 1) * 128],
                        inb[:, bass.ts(h, MM_N)],
                        start=True,
                        stop=True,
                    )

            tiles = [(s, i) for s in range(nslot) for i in range(t_all[s])]
            for k in range(min(LOOK, len(tiles))):
                emit_s(*tiles[k])
            for k, (s, i) in enumerate(tiles):
                t = t_all[s]
                inb = inb_tiles[s]
                vbase = QCH + 128 * t
                scoff = s * 4 * KT
                et = ets[s]
                s_ps = sps_ref.pop((s, i))
                if i in act_sets[s]:
                    nc.scalar.activation(
                        et[:, i, :],
                        s_ps[:],
                        mybir.ActivationFunctionType.Exp,
                        bias=sc_sb[:, scoff + KT + i : scoff + KT + i + 1],
                        scale=sc_sb[:, scoff + i : scoff + i + 1],
                    )
                else:
                    nc.vector.tensor_scalar(
                        et[:, i, :].bitcast(I16),
                        s_ps[:],
                        sc_sb[:, scoff + 2 * KT + i : scoff + 2 * KT + i + 1],
                        sc_sb[:, scoff + 3 * KT + i : scoff + 3 * KT + i + 1],
                        mybir.AluOpType.mult,
                        mybir.AluOpType.add,
                    )
                for h in range(QCH // MM_N):
                    nc.tensor.matmul(
                        o_pss[s][:, bass.ts(h, MM_N)],
                        inb[:, vbase + i * 128 : vbase + (i + 1) * 128],
                        et[:, i, bass.ts(h, MM_N)],
                        start=(i == 0),
                        stop=(i == t - 1),
                    )
                if k + LOOK < len(tiles):
                    emit_s(*tiles[k + LOOK])
                if i == t - 1:
                    # slot epilogue: denominator pair-sums in place (pair j:
                    # e[2j]+e[2j+1] -> e[j]; odd tail ships raw) + batched
                    # esum DMA; then the oT copy and DMA.
                    pe = pair_eng[s]
                    if pe == "g":
                        # ship each half from the gpsimd queue right after
                        # its pairs complete so the big esum transfer
                        # overlaps the remaining compute
                        np_half = (t // 2 + 1) // 2
                        for j in range(t // 2):
                            nc.gpsimd.tensor_add(
                                et[:, j, :], et[:, 2 * j, :], et[:, 2 * j + 1, :]
                            )
                            if j == np_half - 1:
                                nc.gpsimd.dma_start(
                                    out=esums[s][:, :np_half, :],
                                    in_=et[:, :np_half, :],
                                )
                        nc.gpsimd.dma_start(
                            out=esums[s][:, np_half : t // 2, :],
                            in_=et[:, np_half : t // 2, :],
                        )
                        if t % 2:
                            nc.gpsimd.dma_start(
                                out=esums[s][:, t // 2 : t // 2 + 1, :],
                                in_=et[:, t - 1 : t, :],
                            )
                    elif pe == "v":
                        for j in range(t // 2):
                            nc.vector.tensor_add(
                                et[:, j, :], et[:, 2 * j, :], et[:, 2 * j + 1, :]
                            )
                        nc.sync.dma_start(
                            out=esums[s][:, : t // 2, :], in_=et[:, : t // 2, :]
                        )
                        if t % 2:
                            nc.sync.dma_start(
                                out=esums[s][:, t // 2 : t // 2 + 1, :],
                                in_=et[:, t - 1 : t, :],
                            )
                    else:
                        nc.sync.dma_start(out=esums[s][:], in_=et[:])

                    o_sb = small.tile([128, QCH], F16, tag="osb")
                    if s >= nslot - 2:
                        # tail slots: DVE is idle by now, ScalarE is not
                        nc.vector.tensor_copy(o_sb[:], o_pss[s][:])
                        nc.sync.dma_start(out=oT[s], in_=o_sb[:])
                    else:
                        nc.scalar.copy(o_sb[:], o_pss[s][:])
                        nc.scalar.dma_start(out=oT[s], in_=o_sb[:])

    nc.compile()
    return nc


def kernel(q, k, v, valid_lens):
    q = np.ascontiguousarray(q, dtype=np.float32)
    k = np.ascontiguousarray(k, dtype=np.float32)
    v = np.ascontiguousarray(v, dtype=np.float32)
    L = np.asarray(valid_lens).reshape(-1).astype(np.int64)

    # per-batch key-tile need; L==0 batches are handled entirely on the host
    # (uniform softmax over all keys == plain mean of v)
    need = np.minimum(KT, (L + 127) // 128).astype(np.int64)

    needs = [
        (int(need[b]), b, h)
        for b in range(B)
        for h in range(Q // QCH)
        if need[b] > 0
    ]
    sizes, segs = _plan_slots(needs)
    # largest slot first (its GpSimd pairing is serial and slow), smallest
    # last (its drain + esum DMA is the tail)
    order = sorted(range(len(sizes)), key=lambda j: -sizes[j])
    trips = tuple(sizes[j] for j in order)
    assign = [[segs[c][j] for j in order] for c in range(NCORES)]
    # denominator pair-sum engine per slot: GpSimd takes the largest slot
    # (it is otherwise idle; DVE is drain-bound), DVE the rest; slots with
    # one tile ship raw.
    nslot = len(trips)
    big = max(range(nslot), key=lambda s: trips[s])
    pair_eng = tuple(
        "" if trips[s] < 2 else ("g" if s == big else "v") for s in range(nslot)
    )
    # drain split: ScalarE takes ~58% of the key tiles, interleaved with the
    # DVE Schraudolph tiles so both drain engines run concurrently; both
    # paths mask via data vectors, so the split is a pure balance knob.
    # While GpSimd streams its pair-sums it contends with DVE's SBUF port,
    # so the GpSimd-paired slot shifts extra drains onto ScalarE.
    act_sets = tuple(
        frozenset(
            i
            for i in range(t)
            if int((i + 1) * r) > int(i * r)
        )
        for t, r in ((trips[s], 0.62 if pair_eng[s] == "g" else 0.56) for s in range(nslot))
    )

    key = (trips, act_sets, pair_eng)
    if key not in _build_cache:
        _build_cache[key] = _build(trips, act_sets, pair_eng)
    nc = _build_cache[key]

    qh = q.astype(np.float16)
    kh = k.astype(np.float16)
    vh = v.astype(np.float16)

    # Schraudolph scale/bias per (key-tile, partition): for valid keys
    #   t16 = S*(SCALE*log2e*1024) + (15-C)*1024 ; int16(t16) bitcast fp16
    # masked keys get scale=bias=0 -> +0.0 exactly.
    kidx = np.arange(K)
    scE_all = np.zeros((B, 128, KT), np.float32)
    biE_all = np.full((B, 128, KT), -30.0, np.float32)
    sc2_all = np.zeros((B, 128, KT), np.float32)
    bi2_all = np.zeros((B, 128, KT), np.float32)
    svals = np.float32(SCALE * LOG2E * 1024.0)
    bvals = np.float32((15.0 - SCHC) * 1024.0)
    for b in range(B):
        lb = int(L[b])
        if lb == 0:
            continue
        m = (kidx < lb).astype(np.float32)
        scE_all[b] = (m * np.float32(SCALE)).reshape(KT, 128).T
        biE_all[b] = ((1.0 - m) * np.float32(-30.0)).reshape(KT, 128).T
        sc2_all[b] = (m * svals).reshape(KT, 128).T
        bi2_all[b] = (m * bvals).reshape(KT, 128).T

    in_maps = []
    for c in range(NCORES):
        im = {}
        scbi = np.zeros((128, nslot * 4 * KT), np.float32)
        for s in range(nslot):
            t = trips[s]
            seg = assign[c][s]
            pack = np.zeros((128, QCH + 2 * 128 * t), np.float16)
            if seg is not None:
                b, h, off, ln = seg
                pack[:, :QCH] = qh[b, h * QCH : (h + 1) * QCH].T
                k0, k1 = off * 128, (off + ln) * 128
                pack[:, QCH : QCH + 128 * ln] = kh[b, k0:k1].T
                # v permuted: partition = key-within-tile, cols = (tile, d)
                pack[:, QCH + 128 * t : QCH + 128 * (t + ln)] = (
                    vh[b, k0:k1].reshape(ln, 128, D).transpose(1, 0, 2).reshape(128, -1)
                )
                o = s * 4 * KT
                scbi[:, o : o + ln] = scE_all[b][:, off : off + ln]
                scbi[:, o + KT : o + KT + ln] = biE_all[b][:, off : off + ln]
                scbi[:, o + 2 * KT : o + 2 * KT + ln] = sc2_all[b][:, off : off + ln]
                scbi[:, o + 3 * KT : o + 3 * KT + ln] = bi2_all[b][:, off : off + ln]
            # padding tiles (i >= ln) keep scE=0/biE=-30 and sc2=bi2=0 -> E=0
            scbi[:, s * 4 * KT + KT + (0 if seg is None else seg[3]) : s * 4 * KT + 2 * KT] = -30.0
            im[f"inb{s}"] = np.ascontiguousarray(pack)
        im["scbi"] = scbi
        in_maps.append(im)

    res = run_bass_kernel_spmd(nc, in_maps, list(range(NCORES)))
    global last_result
    last_result = res

    num = np.zeros((B, Q // QCH, 128, QCH), np.float32)
    den = np.zeros((B, Q // QCH, QCH), np.float32)
    for c in range(NCORES):
        r = res.results[c]
        for s in range(nslot):
            seg = assign[c][s]
            if seg is None:
                continue
            b, h, off, ln = seg
            num[b, h] += r["oT"][s].astype(np.float32)
            den[b, h] += r[f"esum{s}"].astype(np.float32).sum(axis=(0, 1))
    out = np.empty((B, Q, D), np.float32)
    for b in range(B):
        if L[b] == 0:
            out[b] = v[b].mean(axis=0)[None, :]
            continue
        for h in range(Q // QCH):
            out[b, h * QCH : (h + 1) * QCH] = (num[b, h] / den[b, h][None, :]).T
    return out
